# revision 27
# baseline (speedup 1.0000x reference)
"""CTC loss (T=512, B=32, C=8000, L=2, blank=0) on 8 Trainium2 NeuronCores.

Data-parallel over batch: each core takes 4 sequences. v5 structure:
  - host gathers the 16 needed logit streams per core as one contiguous
    [16, 512] tile X (a, y1, a_rev, y2_rev per sequence) -> tiny DMA
    (no TTR / gpsimd-DMA / strided-scalar outs: those hang this runtime),
  - ONE f32r PE matmul with a +-1 matrix Wm [16, 100] forms the DP
    streams at 32-aligned partition groups: d1 rows 0-3, d2rev rows
    32-35, y1 rows 64-67, a rows 96-99,
  - ONE fused inclusive cumsum into a PSUM-resident PBX [100, 512];
    d2 and a arrive time-reversed, so PBX[32:36, 0:512] IS the suffix
    sum P1brev, PBX[0:4] IS P1a, and rows 96-99 give the a-suffix sums
    (PSUM operands are exempt from the SBUF same-base-partition rule),
  - s1+s3 batched [36,512] as a direct view of PBX,
  - Vector carries only the critical chain (scans / maxes / P2 / Zthr);
    GpSimd carries the ZC/Zskip side chain and OUT staging copies,
  - device ships OUT [36,4] = (SZ, negMz, nm1, nm2); host does the
    final ln, max-unwinding, logaddexp, /L and batch mean in float64.

Notation (per sequence b, t = 0..511):
  a_t = logit[t,b,0], y1_t = logit[t,b,t1], y2_t = logit[t,b,t2]
  P1a_t = sum_{tau<t}(a-y1);  P1brev_c = sum_{t>511-c}(a-y2)
  W1 = ln cumsum exp(P1 - m1) + K;  P2rev_c = W1brev_{c-1} - P1brev_c
  W2 = ln cumsum exp(P2rev - m2) + K
  thr:  Zt_t = ZC_t + W2true_t       skip: Zs_t = ZC_{t+1} + P2true_t
  pcat0 = lnSZt + Mzt + m1a+m1b+m2 - 3K
  pcat1 = lnSZs + Mzs + m1a+m1b - 2K + skip
  loss_b = -logaddexp(pcat0, pcat1)/L
"""
import numpy as np

T = 512
B = 32
C = 8000
L = 2
NCORES = 8
BS = B // NCORES          # 4 sequences per core
XR = 4 * BS               # 16 input stream rows
NP = 100                  # stream partition span (groups at 0/32/64/96)
NZ = 36                   # two-group span (rows 0-3 and 32-35)
NEG = -1e30
EPS = 4.4e-20   # bottom edge of the HW Ln table's accurate range
KLN = 3e16      # scale so S*KLN spans the Ln-accurate domain
KAPPA = float(np.log(3e16))


def build_bass(dbg=False):
    import concourse.bass as bass
    import concourse.bacc as bacc
    import concourse.mybir as mybir
    import concourse.tile as tile
    from contextlib import ExitStack

    f32 = mybir.dt.float32
    f32r = mybir.dt.float32r
    AT = mybir.ActivationFunctionType
    OP = mybir.AluOpType
    AX = mybir.AxisListType

    nc = bacc.Bacc("TRN2", target_bir_lowering=False, debug=False,
                   num_devices=NCORES)

    # Exp and Ln share the natural_log_exp_and_others ACT table set; pin
    # the chooser there so the table loads once (no Exp<->Ln reloads).
    import types
    from concourse.hw_specs import get_activation_tables

    def _act_loads_one_set(self):
        has_activation = any(isinstance(i, mybir.InstActivation)
                             for b in self.main_func.blocks
                             for i in b.instructions)
        if not has_activation:
            return
        tables = [(n, (fns if n == "natural_log_exp_and_others" else set()))
                  for n, fns in get_activation_tables(self.m.arch).items()]
        bacc._bass_rust.insert_act_table_loads(self, tables)

    nc.insert_act_table_loads = types.MethodType(_act_loads_one_set, nc)

    x_ext = nc.dram_tensor("x", [XR, T], f32r, kind="ExternalInput")
    w_ext = nc.dram_tensor("w", [XR, NP], f32r, kind="ExternalInput")
    out_ext = nc.dram_tensor("out", [8, 4], f32, kind="ExternalOutput")

    with tile.TileContext(nc) as tc, ExitStack() as ctx:
        pool = ctx.enter_context(tc.tile_pool(name="p", bufs=1))
        ppool = ctx.enter_context(tc.tile_pool(name="ps", bufs=1, space="PSUM"))

        # ---------- DMAs first (both tiny and contiguous) ----------
        Xs = pool.tile([XR, T], f32r)
        Ws = pool.tile([XR, NP], f32r)
        nc.sync.dma_start(out=Xs[:], in_=x_ext[:])
        nc.scalar.dma_start(out=Ws[:], in_=w_ext[:])

        # ---------- constants + memsets (off critical path) ----------
        zeros = pool.tile([NP, 1], f32)
        nc.gpsimd.memset(zeros[:], 0.0)
        eps36 = pool.tile([NZ, 1], f32)
        nc.gpsimd.memset(eps36[:], EPS)

        PBX = ppool.tile([NP, T], f32, tag="PBX")
        nc.vector.memset(PBX[:, 0:1], 0.0)
        P2 = pool.tile([BS, T], f32)
        nc.gpsimd.memset(P2[:, 0:1], NEG)

        # preload the Exp/Ln ACT table during the DMA window
        warm = pool.tile([1, 1], f32)
        nc.scalar.activation(warm[:], zeros[0:1, :], AT.Exp,
                             bias=eps36[0:1, :], scale=1.0)

        # ---------- phase A: one matmul -> streams [slot, t] ----------
        STR = ppool.tile([NP, T], f32, tag="STR")
        nc.tensor.matmul(STR[:], Ws[:], Xs[:], start=True, stop=True)

        # ---------- phase B: ONE fused inclusive cumsum ----------
        # PBX[r, t+1] = sum_{tau<=t} STR[r, tau]; col 0 = 0.
        # Rows 0-3: P1a = PBX[0:4, 0:512] (exclusive-view). Rows 32-35:
        # d2 arrives time-reversed, so P1brev = PBX[32:36, 0:512].
        nc.vector.tensor_tensor_scan(
            PBX[:, 1:T], STR[:, 0:T - 1],
            zeros[:].broadcast_to((NP, T - 1)), 0.0,
            op0=OP.add, op1=OP.bypass)
        P1v = PBX[0:NZ, 0:T]

        # ---------- stage s1 (rows 0-3) + s3 (rows 32-35, rev) ----------
        OUT = pool.tile([NZ, 4], f32)
        nm1 = pool.tile([NZ, 1], f32)
        nc.vector.tensor_reduce(nm1[:], P1v, axis=AX.X, op=OP.max,
                                negate=True)
        E1 = ppool.tile([NZ, T], f32, tag="E1")
        nc.scalar.activation(E1[:], P1v, AT.Exp, bias=nm1[:], scale=1.0)
        # PBX is PSUM: GpSimd cannot read it, so TY1z / ZCp run on
        # Vector inside the E1-exp and W1-ln windows; TAs_t (suffix-sum
        # of a) comes from the reversed a_rev cumsum rows 96-99.
        TY1z = pool.tile([BS, T], f32)    # TY1e at base partition 0
        ZCp = pool.tile([BS, T], f32)     # TY1e_t + TAs_t
        nc.scalar.activation(TY1z[:], PBX[64:68, 0:T], AT.Copy)
        nc.gpsimd.tensor_copy(OUT[:, 2:3], nm1[:])
        S1 = pool.tile([NZ, T], f32)
        nc.vector.tensor_tensor_scan(S1[:], E1[:],
                                     zeros[0:NZ, :].broadcast_to((NZ, T)),
                                     0.0, op0=OP.add, op1=OP.bypass)
        nc.vector.tensor_tensor(ZCp[:, 1:T], TY1z[:, 1:T],
                                PBX[96:100, 1:T][:, ::-1], op=OP.add)
        W1 = pool.tile([NZ, T], f32)      # W' = true W + KAPPA
        nc.scalar.activation(W1[:], S1[:], AT.Ln, bias=eps36[:], scale=KLN)

        # ---------- stage s2 (rev) ----------
        nc.vector.tensor_tensor(P2[:, 1:T], W1[32:36, 0:T - 1],
                                PBX[32:36, 1:T], op=OP.subtract)
        nm2 = pool.tile([BS, 1], f32)
        nc.vector.tensor_reduce(nm2[:], P2[:], axis=AX.X, op=OP.max,
                                negate=True)
        E2 = ppool.tile([BS, T], f32, tag="E2")
        nc.scalar.activation(E2[:], P2[:], AT.Exp, bias=nm2[:], scale=1.0)
        # ZC / Zskip side chain on GpSimd under the s2 windows
        ZC = pool.tile([BS, T], f32)      # ZC_t = TY1e_t + TAs_t + W1a_{t-1}
        Zs = pool.tile([BS, T], f32)      # skip half: ZC_{t+1} + P2true_t
        nc.gpsimd.tensor_tensor(ZC[:, 1:T], ZCp[:, 1:T],
                                W1[0:BS, 0:T - 1], op=OP.add)
        nc.gpsimd.tensor_tensor(Zs[:, 0:T - 1], ZC[:, 1:T],
                                P2[:, 1:T][:, ::-1], op=OP.add)
        nc.gpsimd.tensor_copy(OUT[0:BS, 3:4], nm2[:])
        # exp(ZC - mzc) overlaps the S2 scan; the thr-side LSE is then a
        # dot with S2 reversed (exp(W2true) == S2 exactly), so the third
        # Ln plus the batched Zthr/max/exp tail all leave the chain.
        negmzc = pool.tile([BS, 1], f32)
        ECZ = ppool.tile([BS, T - 1], f32, tag="ECZ")
        with tc.high_priority(offset=-10000):
            nc.vector.tensor_reduce(negmzc[:], ZC[:, 1:T], axis=AX.X,
                                    op=OP.max, negate=True)
            nc.scalar.activation(ECZ[:], ZC[:, 1:T], AT.Exp, bias=negmzc[:],
                                 scale=1.0)
            nc.gpsimd.tensor_copy(OUT[0:BS, 1:2], negmzc[:])
        S2 = pool.tile([BS, T], f32)
        nc.vector.tensor_tensor_scan(S2[:], E2[:],
                                     zeros[0:BS, :].broadcast_to((BS, T)),
                                     0.0, op0=OP.add, op1=OP.bypass)

        # ---------- combine ----------
        negMzs = pool.tile([BS, 1], f32)
        with tc.high_priority(offset=-10000):
            nc.vector.tensor_reduce(negMzs[:], Zs[:, 0:T - 1], axis=AX.X,
                                    op=OP.max, negate=True)
        TMPD = pool.tile([BS, T - 1], f32)
        nc.vector.tensor_tensor(TMPD[:], ECZ[:],
                                S2[:, 0:T - 1][:, ::-1], op=OP.mult)
        nc.gpsimd.tensor_copy(OUT[32:36, 1:2], negMzs[:])
        EZs = ppool.tile([BS, T - 1], f32, tag="EZs")
        nc.scalar.activation(EZs[:], Zs[:, 0:T - 1], AT.Exp, bias=negMzs[:],
                             scale=1.0, accum_out=OUT[32:36, 0:1])
        nc.vector.tensor_reduce(OUT[0:BS, 0:1], TMPD[:], axis=AX.X,
                                op=OP.add)
        nc.sync.dma_start(out=out_ext[0:4, :], in_=OUT[0:BS, :])
        nc.scalar.dma_start(out=out_ext[4:8, :], in_=OUT[32:36, :])

    nc.compile()
    return nc


def make_in_maps(logit, targets):
    logit = np.asarray(logit, dtype=np.float32)
    targets = np.asarray(targets)
    in_maps = []
    for core in range(NCORES):
        tg = targets[core * BS:(core + 1) * BS]
        x = np.empty((XR, T), np.float32)
        for b in range(BS):
            gb = core * BS + b
            x[0 + b] = logit[:, gb, 0]                      # a
            x[BS + b] = logit[:, gb, int(tg[b, 0])]         # y1
            x[2 * BS + b] = logit[::-1, gb, 0]              # a reversed
            x[3 * BS + b] = logit[::-1, gb, int(tg[b, 1])]  # y2 reversed
        # group 96 stream switches to the REVERSED a (suffix sums)
        w = np.zeros((XR, NP), np.float32)
        for b in range(BS):
            w[0 + b, 0 + b] = 1.0        # d1 = a - y1
            w[BS + b, 0 + b] = -1.0
            w[2 * BS + b, 32 + b] = 1.0  # d2rev = a_rev - y2_rev
            w[3 * BS + b, 32 + b] = -1.0
            w[BS + b, 64 + b] = 1.0      # y1
            w[2 * BS + b, 96 + b] = 1.0  # a_rev (suffix sums)
        in_maps.append({"x": x, "w": w})
    return in_maps


def finish(results, targets):
    """Host gather: per-core OUT [36,4] -> per-seq losses [32] (float64)."""
    targets = np.asarray(targets)
    losses = np.empty(B, np.float64)
    for core, r in enumerate(results):
        o = np.asarray(r["out"], np.float64)     # [8, 4]
        sz_t, sz_s = o[0:BS, 0], o[BS:2 * BS, 0]
        mz_t, mz_s = -o[0:BS, 1], -o[BS:2 * BS, 1]
        m1a, m1b = -o[0:BS, 2], -o[BS:2 * BS, 2]
        m2 = -o[0:BS, 3]
        tg = targets[core * BS:(core + 1) * BS]
        skip = np.where(tg[:, 0] != tg[:, 1], 0.0, NEG)
        pcat0 = np.log(sz_t) + mz_t + m1a + m1b + m2 - 2 * KAPPA
        pcat1 = np.log(sz_s) + mz_s + m1a + m1b - 2 * KAPPA + skip
        losses[core * BS:(core + 1) * BS] = \
            -np.logaddexp(pcat0, pcat1) / L
    return losses


_CACHED = {}


def kernel(logit, label, targets):
    from concourse.bass_utils import run_bass_kernel_spmd
    if "nc" not in _CACHED:
        _CACHED["nc"] = build_bass()
    nc = _CACHED["nc"]
    in_maps = make_in_maps(logit, targets)
    res = run_bass_kernel_spmd(nc, in_maps, core_ids=list(range(NCORES)))
    losses = finish(res.results, targets)
    return np.float32(losses.mean())


# revision 28
# speedup vs baseline: 1.0100x; 1.0100x over previous
"""CTC loss (T=512, B=32, C=8000, L=2, blank=0) on 8 Trainium2 NeuronCores.

Data-parallel over batch: each core takes 4 sequences. v5 structure:
  - host gathers the 16 needed logit streams per core as one contiguous
    [16, 512] tile X (a, y1, a_rev, y2_rev per sequence) -> tiny DMA
    (no TTR / gpsimd-DMA / strided-scalar outs: those hang this runtime),
  - ONE f32r PE matmul with a +-1 matrix Wm [16, 100] forms the DP
    streams at 32-aligned partition groups: d1 rows 0-3, d2rev rows
    32-35, y1 rows 64-67, a rows 96-99,
  - ONE fused inclusive cumsum into a PSUM-resident PBX [100, 512];
    d2 and a arrive time-reversed, so PBX[32:36, 0:512] IS the suffix
    sum P1brev, PBX[0:4] IS P1a, and rows 96-99 give the a-suffix sums
    (PSUM operands are exempt from the SBUF same-base-partition rule),
  - s1+s3 batched [36,512] as a direct view of PBX,
  - Vector carries only the critical chain (scans / maxes / P2 / Zthr);
    GpSimd carries the ZC/Zskip side chain and OUT staging copies,
  - device ships OUT [36,4] = (SZ, negMz, nm1, nm2); host does the
    final ln, max-unwinding, logaddexp, /L and batch mean in float64.

Notation (per sequence b, t = 0..511):
  a_t = logit[t,b,0], y1_t = logit[t,b,t1], y2_t = logit[t,b,t2]
  P1a_t = sum_{tau<t}(a-y1);  P1brev_c = sum_{t>511-c}(a-y2)
  W1 = ln cumsum exp(P1 - m1) + K;  P2rev_c = W1brev_{c-1} - P1brev_c
  W2 = ln cumsum exp(P2rev - m2) + K
  thr:  Zt_t = ZC_t + W2true_t       skip: Zs_t = ZC_{t+1} + P2true_t
  pcat0 = lnSZt + Mzt + m1a+m1b+m2 - 3K
  pcat1 = lnSZs + Mzs + m1a+m1b - 2K + skip
  loss_b = -logaddexp(pcat0, pcat1)/L
"""
import numpy as np

T = 512
B = 32
C = 8000
L = 2
NCORES = 8
BS = B // NCORES          # 4 sequences per core
XR = 4 * BS               # 16 input stream rows
NP = 100                  # stream partition span (groups at 0/32/64/96)
NZ = 36                   # two-group span (rows 0-3 and 32-35)
NEG = -1e30
EPS = 4.4e-20   # bottom edge of the HW Ln table's accurate range
KLN = 3e16      # scale so S*KLN spans the Ln-accurate domain
KAPPA = float(np.log(3e16))


def build_bass(dbg=False):
    import concourse.bass as bass
    import concourse.bacc as bacc
    import concourse.mybir as mybir
    import concourse.tile as tile
    from contextlib import ExitStack

    f32 = mybir.dt.float32
    f32r = mybir.dt.float32r
    AT = mybir.ActivationFunctionType
    OP = mybir.AluOpType
    AX = mybir.AxisListType

    nc = bacc.Bacc("TRN2", target_bir_lowering=False, debug=False,
                   num_devices=NCORES)

    # Exp and Ln share the natural_log_exp_and_others ACT table set; pin
    # the chooser there so the table loads once (no Exp<->Ln reloads).
    import types
    from concourse.hw_specs import get_activation_tables

    def _act_loads_one_set(self):
        has_activation = any(isinstance(i, mybir.InstActivation)
                             for b in self.main_func.blocks
                             for i in b.instructions)
        if not has_activation:
            return
        tables = [(n, (fns if n == "natural_log_exp_and_others" else set()))
                  for n, fns in get_activation_tables(self.m.arch).items()]
        bacc._bass_rust.insert_act_table_loads(self, tables)

    nc.insert_act_table_loads = types.MethodType(_act_loads_one_set, nc)

    x_ext = nc.dram_tensor("x", [XR, T], f32r, kind="ExternalInput")
    w_ext = nc.dram_tensor("w", [XR, NP], f32r, kind="ExternalInput")
    out_ext = nc.dram_tensor("out", [8, 4], f32, kind="ExternalOutput")

    with tile.TileContext(nc) as tc, ExitStack() as ctx:
        pool = ctx.enter_context(tc.tile_pool(name="p", bufs=1))
        ppool = ctx.enter_context(tc.tile_pool(name="ps", bufs=1, space="PSUM"))

        # ---------- DMAs first (both tiny and contiguous) ----------
        Xs = pool.tile([XR, T], f32r)
        Ws = pool.tile([XR, NP], f32r)
        nc.sync.dma_start(out=Xs[:], in_=x_ext[:])
        nc.scalar.dma_start(out=Ws[:], in_=w_ext[:])

        # ---------- constants + memsets (off critical path) ----------
        zeros = pool.tile([NP, 1], f32)
        nc.gpsimd.memset(zeros[:], 0.0)
        eps36 = pool.tile([NZ, 1], f32)
        nc.gpsimd.memset(eps36[:], EPS)

        PBX = ppool.tile([NP, T], f32, tag="PBX")
        nc.vector.memset(PBX[:, 0:1], 0.0)
        P2 = pool.tile([BS, T], f32)
        nc.gpsimd.memset(P2[:, 0:1], NEG)

        # preload the Exp/Ln ACT table during the DMA window
        warm = pool.tile([1, 1], f32)
        nc.scalar.activation(warm[:], zeros[0:1, :], AT.Exp,
                             bias=eps36[0:1, :], scale=1.0)

        # ---------- phase A: one matmul -> streams [slot, t] ----------
        STR = ppool.tile([NP, T], f32, tag="STR")
        nc.tensor.matmul(STR[:], Ws[:], Xs[:], start=True, stop=True)

        # ---------- phase B: ONE fused inclusive cumsum ----------
        # PBX[r, t+1] = sum_{tau<=t} STR[r, tau]; col 0 = 0.
        # Rows 0-3: P1a = PBX[0:4, 0:512] (exclusive-view). Rows 32-35:
        # d2 arrives time-reversed, so P1brev = PBX[32:36, 0:512].
        nc.vector.tensor_tensor_scan(
            PBX[:, 1:T], STR[:, 0:T - 1],
            zeros[:].broadcast_to((NP, T - 1)), 0.0,
            op0=OP.add, op1=OP.bypass)
        P1v = PBX[0:NZ, 0:T]

        # ---------- stage s1 (rows 0-3) + s3 (rows 32-35, rev) ----------
        OUT = pool.tile([NZ, 4], f32)
        nm1 = pool.tile([NZ, 1], f32)
        nc.vector.tensor_reduce(nm1[:], P1v, axis=AX.X, op=OP.max,
                                negate=True)
        E1 = ppool.tile([NZ, T], f32, tag="E1")
        nc.scalar.activation(E1[:], P1v, AT.Exp, bias=nm1[:], scale=1.0)
        # PBX is PSUM: GpSimd cannot read it, so TY1z / ZCp run on
        # Vector inside the E1-exp and W1-ln windows; TAs_t (suffix-sum
        # of a) comes from the reversed a_rev cumsum rows 96-99.
        TY1z = pool.tile([BS, T], f32)    # TY1e at base partition 0
        ZCp = pool.tile([BS, T], f32)     # TY1e_t + TAs_t
        nc.scalar.activation(TY1z[:], PBX[64:68, 0:T], AT.Copy)
        nc.gpsimd.tensor_copy(OUT[:, 2:3], nm1[:])
        S1 = pool.tile([NZ, T], f32)
        nc.vector.tensor_tensor_scan(S1[:], E1[:],
                                     zeros[0:NZ, :].broadcast_to((NZ, T)),
                                     0.0, op0=OP.add, op1=OP.bypass)
        nc.vector.tensor_tensor(ZCp[:, 1:T], TY1z[:, 1:T],
                                PBX[96:100, 1:T][:, ::-1], op=OP.add)
        W1 = pool.tile([NZ, T], f32)      # W' = true W + KAPPA
        nc.scalar.activation(W1[:], S1[:], AT.Ln, bias=eps36[:], scale=KLN)

        # ---------- stage s2 (rev) ----------
        nc.vector.tensor_tensor(P2[:, 1:T], W1[32:36, 0:T - 1],
                                PBX[32:36, 1:T], op=OP.subtract)
        nm2 = pool.tile([BS, 1], f32)
        nc.vector.tensor_reduce(nm2[:], P2[:], axis=AX.X, op=OP.max,
                                negate=True)
        E2 = ppool.tile([BS, T], f32, tag="E2")
        nc.scalar.activation(E2[:], P2[:], AT.Exp, bias=nm2[:], scale=1.0)
        # ZC / Zskip side chain on GpSimd under the s2 windows
        ZC = pool.tile([BS, T], f32)      # ZC_t = TY1e_t + TAs_t + W1a_{t-1}
        Zs = pool.tile([BS, T], f32)      # skip half: ZC_{t+1} + P2true_t
        nc.gpsimd.tensor_tensor(ZC[:, 1:T], ZCp[:, 1:T],
                                W1[0:BS, 0:T - 1], op=OP.add)
        nc.gpsimd.tensor_tensor(Zs[:, 0:T - 1], ZC[:, 1:T],
                                P2[:, 1:T][:, ::-1], op=OP.add)
        nc.gpsimd.tensor_copy(OUT[0:BS, 3:4], nm2[:])
        # exp(ZC - mzc) overlaps the S2 scan; the thr-side LSE is then a
        # dot with S2 reversed (exp(W2true) == S2 exactly), so the third
        # Ln plus the batched Zthr/max/exp tail all leave the chain.
        negmzc = pool.tile([BS, 1], f32)
        ECZ = ppool.tile([BS, T - 1], f32, tag="ECZ")
        with tc.high_priority(offset=10000):
            nc.vector.tensor_reduce(negmzc[:], ZC[:, 1:T], axis=AX.X,
                                    op=OP.max, negate=True)
            nc.scalar.activation(ECZ[:], ZC[:, 1:T], AT.Exp, bias=negmzc[:],
                                 scale=1.0)
            nc.gpsimd.tensor_copy(OUT[0:BS, 1:2], negmzc[:])
        S2 = pool.tile([BS, T], f32)
        nc.vector.tensor_tensor_scan(S2[:], E2[:],
                                     zeros[0:BS, :].broadcast_to((BS, T)),
                                     0.0, op0=OP.add, op1=OP.bypass)

        # ---------- combine ----------
        negMzs = pool.tile([BS, 1], f32)
        with tc.high_priority(offset=10000):
            nc.vector.tensor_reduce(negMzs[:], Zs[:, 0:T - 1], axis=AX.X,
                                    op=OP.max, negate=True)
        TMPD = pool.tile([BS, T - 1], f32)
        nc.vector.tensor_tensor(TMPD[:], ECZ[:],
                                S2[:, 0:T - 1][:, ::-1], op=OP.mult)
        nc.gpsimd.tensor_copy(OUT[32:36, 1:2], negMzs[:])
        EZs = ppool.tile([BS, T - 1], f32, tag="EZs")
        nc.scalar.activation(EZs[:], Zs[:, 0:T - 1], AT.Exp, bias=negMzs[:],
                             scale=1.0, accum_out=OUT[32:36, 0:1])
        nc.vector.tensor_reduce(OUT[0:BS, 0:1], TMPD[:], axis=AX.X,
                                op=OP.add)
        nc.sync.dma_start(out=out_ext[0:4, :], in_=OUT[0:BS, :])
        nc.scalar.dma_start(out=out_ext[4:8, :], in_=OUT[32:36, :])

    nc.compile()
    return nc


def make_in_maps(logit, targets):
    logit = np.asarray(logit, dtype=np.float32)
    targets = np.asarray(targets)
    in_maps = []
    for core in range(NCORES):
        tg = targets[core * BS:(core + 1) * BS]
        x = np.empty((XR, T), np.float32)
        for b in range(BS):
            gb = core * BS + b
            x[0 + b] = logit[:, gb, 0]                      # a
            x[BS + b] = logit[:, gb, int(tg[b, 0])]         # y1
            x[2 * BS + b] = logit[::-1, gb, 0]              # a reversed
            x[3 * BS + b] = logit[::-1, gb, int(tg[b, 1])]  # y2 reversed
        # group 96 stream switches to the REVERSED a (suffix sums)
        w = np.zeros((XR, NP), np.float32)
        for b in range(BS):
            w[0 + b, 0 + b] = 1.0        # d1 = a - y1
            w[BS + b, 0 + b] = -1.0
            w[2 * BS + b, 32 + b] = 1.0  # d2rev = a_rev - y2_rev
            w[3 * BS + b, 32 + b] = -1.0
            w[BS + b, 64 + b] = 1.0      # y1
            w[2 * BS + b, 96 + b] = 1.0  # a_rev (suffix sums)
        in_maps.append({"x": x, "w": w})
    return in_maps


def finish(results, targets):
    """Host gather: per-core OUT [36,4] -> per-seq losses [32] (float64)."""
    targets = np.asarray(targets)
    losses = np.empty(B, np.float64)
    for core, r in enumerate(results):
        o = np.asarray(r["out"], np.float64)     # [8, 4]
        sz_t, sz_s = o[0:BS, 0], o[BS:2 * BS, 0]
        mz_t, mz_s = -o[0:BS, 1], -o[BS:2 * BS, 1]
        m1a, m1b = -o[0:BS, 2], -o[BS:2 * BS, 2]
        m2 = -o[0:BS, 3]
        tg = targets[core * BS:(core + 1) * BS]
        skip = np.where(tg[:, 0] != tg[:, 1], 0.0, NEG)
        pcat0 = np.log(sz_t) + mz_t + m1a + m1b + m2 - 2 * KAPPA
        pcat1 = np.log(sz_s) + mz_s + m1a + m1b - 2 * KAPPA + skip
        losses[core * BS:(core + 1) * BS] = \
            -np.logaddexp(pcat0, pcat1) / L
    return losses


_CACHED = {}


def kernel(logit, label, targets):
    from concourse.bass_utils import run_bass_kernel_spmd
    if "nc" not in _CACHED:
        _CACHED["nc"] = build_bass()
    nc = _CACHED["nc"]
    in_maps = make_in_maps(logit, targets)
    res = run_bass_kernel_spmd(nc, in_maps, core_ids=list(range(NCORES)))
    losses = finish(res.results, targets)
    return np.float32(losses.mean())
